# revision 5
# baseline (speedup 1.0000x reference)
"""Causal GQA self-attention with RoPE for TRN2, 8 NeuronCores.

Problem: B=2, S=2048, D=2048, H=16 q-heads, KV=4 kv-heads, HD=128.

Sharding: core c = (batch b = c//4, kv-group g = c%4). Each core computes
q-heads 4g..4g+3 and kv-head g for batch b:
  qT = (x[b] @ wq[:, 512g:512g+512]).T      via PE, transposed layout
  kT/vT likewise (128-wide slices), RoPE on qT/kT,
  flash-style causal attention in the S^T (keys-on-partitions) layout,
  partial^T = (attn @ wo[512g:512g+512, :]).T
Host sums the 4 partials per batch and transposes back.

All matmuls run in float32r (TF32-like, full PE rate); softmax math fp32.
"""
import sys

sys.path.insert(0, "/opt/trn_rl_repo")

import numpy as np

import concourse.bass as bass
import concourse.tile as tile
from concourse import bacc, mybir
from concourse.bass_utils import run_bass_kernel_spmd

F32 = mybir.dt.float32
F32R = mybir.dt.float32r
AF = mybir.ActivationFunctionType
OP = mybir.AluOpType

P = 128          # partitions / head dim
S = 2048         # sequence length
D = 2048         # model dim
NH = 4           # q heads per core
QW = NH * P      # q projection width per core (512)
NKD = D // P     # contraction chunks (16)
QCH = 512        # query chunk (free dim of attention matmuls)
NQC = S // QCH   # 4
KCH = P          # key chunk (128, on partitions)
NKC = S // KCH   # 16
SCALE = float(P) ** -0.5


def _host_constants():
    inv = 1.0 / (10000.0 ** (np.arange(0, P, 2, dtype=np.float64) / P))  # [64]
    pos = np.arange(S, dtype=np.float64)
    freqs = pos[:, None] * inv[None, :]                  # [S, 64]
    emb = np.concatenate([freqs, freqs], axis=-1)        # [S, 128]
    cosT = np.cos(emb).T.astype(np.float32).copy()       # [128, S]
    sinT = np.sin(emb).T.astype(np.float32).copy()
    sinT[: P // 2] *= -1.0                               # fold rotate_half sign
    # causal step masks: mask[p, j, q] = 1 if q >= p + 128*j
    q = np.arange(QCH)[None, None, :]
    p = np.arange(P)[:, None, None]
    j = np.arange(4)[None, :, None]
    masks = (q >= p + KCH * j).astype(np.float32)        # [128, 4, 512]
    ident = np.eye(P, dtype=np.float32)
    ones = np.ones((P, 1), dtype=np.float32)
    return cosT, sinT, masks, ident, ones


def build_nc():
    cosT_np, sinT_np, masks_np, ident_np, ones_np = _host_constants()

    nc = bacc.Bacc(None)
    xT_d = nc.dram_tensor("xT", [D, S], F32R, kind="ExternalInput")
    wq_d = nc.dram_tensor("wq", [D, QW], F32R, kind="ExternalInput")
    wk_d = nc.dram_tensor("wk", [D, P], F32R, kind="ExternalInput")
    wv_d = nc.dram_tensor("wv", [D, P], F32R, kind="ExternalInput")
    wo_d = nc.dram_tensor("wo", [QW, D], F32R, kind="ExternalInput")
    out_d = nc.dram_tensor("outT", [D, S], F32, kind="ExternalOutput")

    cos_d = nc.inline_tensor(cosT_np, name="cosT")
    sin_d = nc.inline_tensor(sinT_np, name="sinT")
    mask_d = nc.inline_tensor(masks_np, name="masks")
    ident_d = nc.inline_tensor(ident_np, name="ident")
    ones_d = nc.inline_tensor(ones_np, name="onesv")

    # DRAM views with the contraction dim split for SBUF partitions.
    xT_v = xT_d[:].rearrange("(kd p) s -> p kd s", p=P)
    wq_v = wq_d[:].rearrange("(kd p) c -> p kd c", p=P)
    wk_v = wk_d[:].rearrange("(kd p) c -> p kd c", p=P)
    wv_v = wv_d[:].rearrange("(kd p) c -> p kd c", p=P)
    wo_v = wo_d[:].rearrange("(a p) o -> p a o", p=P)

    with tile.TileContext(nc) as tc:
        with tc.tile_pool(name="persist", bufs=1) as pp:
            qT = pp.tile([P, NH, S], F32R)       # q^T (becomes rope'd)
            kT = pp.tile([P, S], F32R)
            vT = pp.tile([P, S], F32R)
            vK = pp.tile([P, NKC, P], F32R)      # V as (kpos, kchunk, hd)
            mask_t = pp.tile([P, 4, QCH], F32R)
            ones_t = pp.tile([P, 1], F32R)
            ident_t = pp.tile([P, P], F32R)
            nc.sync.dma_start(mask_t[:], mask_d[:].bitcast(F32R))
            nc.sync.dma_start(ones_t[:], ones_d[:].bitcast(F32R))
            nc.sync.dma_start(ident_t[:], ident_d[:].bitcast(F32R))

            # attention output overwrites qT in place: slice (h, jq-chunk) is
            # written only after every read of that same slice is done.
            attnT = qT

            # ---------------- Phase 1: QKV projections -----------------
            with tc.tile_pool(name="xqp", bufs=2) as xqp, \
                 tc.tile_pool(name="p1", bufs=2) as p1, \
                 tc.tile_pool(name="p1c", bufs=1) as p1c, \
                 tc.tile_pool(name="psA", bufs=1, space="PSUM") as psA:
                cos_t = p1c.tile([P, S], F32R)
                sin_t = p1c.tile([P, S], F32R)
                nc.sync.dma_start(cos_t[:], cos_d[:].bitcast(F32R))
                nc.sync.dma_start(sin_t[:], sin_d[:].bitcast(F32R))

                # cc order: k, v, then q heads — k/v ready earliest.
                CC = [("k", 0), ("v", 0), ("q", 0), ("q", 1), ("q", 2), ("q", 3)]
                for iq in range(NQC):            # 512-row quarters
                    xq = xqp.tile([P, NKD, QCH], F32R, tag="xq")
                    for kd in range(NKD):
                        nc.sync.dma_start(
                            xq[:, kd, :], xT_v[:, kd, iq * QCH : (iq + 1) * QCH]
                        )
                    for icc, (kind, hh) in enumerate(CC):
                        if kind == "q":
                            w_v = wq_v[:, :, hh * P : (hh + 1) * P]
                        elif kind == "k":
                            w_v = wk_v
                        else:
                            w_v = wv_v
                        wt = p1.tile([P, NKD, P], F32R, tag="wt")
                        nc.sync.dma_start(wt[:], w_v)
                        ps = psA.tile([P, QCH], F32, tag=f"proj{icc % 2}")
                        for kd in range(NKD):
                            nc.tensor.matmul(
                                ps[:],
                                wt[:, kd, :],
                                xq[:, kd, :],
                                start=(kd == 0),
                                stop=(kd == NKD - 1),
                            )
                        if kind == "q":
                            dst = qT[:, hh, iq * QCH : (iq + 1) * QCH]
                        elif kind == "k":
                            dst = kT[:, iq * QCH : (iq + 1) * QCH]
                        else:
                            dst = vT[:, iq * QCH : (iq + 1) * QCH]
                        nc.scalar.copy(out=dst, in_=ps[:])

                # ---------------- RoPE (k then per-head q) --------------
                def rope(dst_ap):
                    sw = p1.tile([P, S], F32R, tag="swap")
                    half = P // 2
                    nc.sync.dma_start(sw[:half, :], dst_ap[half:, :])
                    nc.sync.dma_start(sw[half:, :], dst_ap[:half, :])
                    nc.vector.tensor_tensor(sw[:], sw[:], sin_t[:], OP.mult)
                    nc.vector.tensor_tensor(dst_ap, dst_ap, cos_t[:], OP.mult)
                    nc.vector.tensor_tensor(dst_ap, dst_ap, sw[:], OP.add)

                rope(kT[:])
                for hh in range(NH):
                    rope(qT[:, hh, :])

                # ---------------- V transpose ---------------------------
                for kc in range(NKC):
                    pt = psA.tile([P, P], F32R, tag=f"proj{kc % 2}")
                    nc.tensor.transpose(
                        pt[:], vT[:, kc * P : (kc + 1) * P], ident_t[:]
                    )
                    nc.vector.tensor_copy(out=vK[:, kc, :], in_=pt[:])

                # ---------------- Phase 2: attention --------------------
                with tc.tile_pool(name="p2", bufs=4) as p2, \
                     tc.tile_pool(name="p2b", bufs=2) as p2b:
                    for h in range(NH):
                        for jq in range(NQC):
                            nkc = 4 * (jq + 1)
                            qs = qT[:, h, jq * QCH : (jq + 1) * QCH]
                            ops = psA.tile([P, QCH], F32, tag=f"o{jq % 2}")
                            acc = p2b.tile([P, QCH], F32R, tag="acc")
                            for kc in range(nkc):
                                sps = psA.tile([P, QCH], F32, tag=f"s{kc % 2}")
                                nc.tensor.matmul(
                                    sps[:],
                                    kT[:, kc * P : (kc + 1) * P],
                                    qs,
                                    start=True,
                                    stop=True,
                                )
                                pT = p2.tile([P, QCH], F32R, tag="pT")
                                nc.scalar.activation(
                                    pT[:], sps[:], AF.Exp, scale=SCALE
                                )
                                jdiag = kc - 4 * jq
                                if jdiag >= 0:
                                    nc.vector.tensor_tensor(
                                        pT[:], pT[:], mask_t[:, jdiag, :], OP.mult
                                    )
                                if kc == 0:
                                    nc.vector.tensor_copy(out=acc[:], in_=pT[:])
                                else:
                                    nc.vector.tensor_tensor(
                                        acc[:], acc[:], pT[:], OP.add
                                    )
                                nc.tensor.matmul(
                                    ops[:],
                                    vK[:, kc, :],
                                    pT[:],
                                    start=(kc == 0),
                                    stop=(kc == nkc - 1),
                                )
                            dps = psA.tile([1, QCH], F32, tag="d")
                            nc.tensor.matmul(
                                dps[:], ones_t[:], acc[:], start=True, stop=True
                            )
                            dinv = p2.tile([1, QCH], F32, tag="dinv")
                            nc.vector.reciprocal(dinv[:], dps[:])
                            dib = p2.tile([P, QCH], F32, tag="dib")
                            nc.gpsimd.partition_broadcast(dib[:], dinv[:])
                            nc.vector.tensor_tensor(
                                attnT[:, h, jq * QCH : (jq + 1) * QCH],
                                ops[:],
                                dib[:],
                                OP.mult,
                            )

            # ---------------- Phase 3: output projection ----------------
            with tc.tile_pool(name="p3", bufs=4) as p3, \
                 tc.tile_pool(name="p3w", bufs=1) as p3w, \
                 tc.tile_pool(name="psB", bufs=1, space="PSUM") as psB:
                wo_t = p3w.tile([P, NH, D], F32R)
                nc.sync.dma_start(wo_t[:], wo_v)
                for oc in range(D // P):
                    pos = [
                        psB.tile([P, QCH], F32, tag=f"po{jq}", name=f"po{jq}")
                        for jq in range(NQC)
                    ]
                    for a in range(NH):
                        for jq in range(NQC):
                            nc.tensor.matmul(
                                pos[jq][:],
                                wo_t[:, a, oc * P : (oc + 1) * P],
                                attnT[:, a, jq * QCH : (jq + 1) * QCH],
                                start=(a == 0),
                                stop=(a == NH - 1),
                            )
                    for jq in range(NQC):
                        ot = p3.tile([P, QCH], F32, tag="ot")
                        nc.scalar.copy(out=ot[:], in_=pos[jq][:])
                        nc.sync.dma_start(
                            out_d[oc * P : (oc + 1) * P, jq * QCH : (jq + 1) * QCH],
                            ot[:],
                        )

    nc.finalize()
    return nc


_NC = None


def _get_nc():
    global _NC
    if _NC is None:
        _NC = build_nc()
    return _NC


def kernel(x, wq, wk, wv, wo):
    x = np.asarray(x, dtype=np.float32)
    wq = np.asarray(wq, dtype=np.float32)
    wk = np.asarray(wk, dtype=np.float32)
    wv = np.asarray(wv, dtype=np.float32)
    wo = np.asarray(wo, dtype=np.float32)

    nc = _get_nc()
    in_maps = []
    for c in range(8):
        b, g = c // 4, c % 4
        in_maps.append(
            {
                "xT": np.ascontiguousarray(x[b].T),
                "wq": np.ascontiguousarray(wq[:, QW * g : QW * (g + 1)]),
                "wk": np.ascontiguousarray(wk[:, P * g : P * (g + 1)]),
                "wv": np.ascontiguousarray(wv[:, P * g : P * (g + 1)]),
                "wo": np.ascontiguousarray(wo[QW * g : QW * (g + 1), :]),
            }
        )
    res = run_bass_kernel_spmd(nc, in_maps, list(range(8)))
    parts = [res.results[c]["outT"] for c in range(8)]
    out = np.stack(
        [
            (parts[0] + parts[1] + parts[2] + parts[3]).T,
            (parts[4] + parts[5] + parts[6] + parts[7]).T,
        ]
    ).astype(np.float32)
    return out
